# revision 7
# baseline (speedup 1.0000x reference)
"""AnchorGenerator on 8 TRN2 NeuronCores.

The reference output depends only on H=W=512 (feature_map values are unused):
for each (y, x, s, r) the anchor row is
    [max(16x+8-hw, 0), max(16y+8-hh, 0), min(16x+8+hw, 8192), min(16y+8+hh, 8192)]
with hw/hh the 3x3 half-width/height tables.

Sharding: 64 grid rows per core. Per core the flat (294912, 4) output slab
is exactly a [128, 9216] SBUF tile in partition-major order, with
partition p = (y_rel, x_half) and free index f = x_rel*36 + (s*3+r)*4 + c.

The unclamped value decomposes as a low-rank product
    v[p, f] = C[f] + Ygrid[p]*my[f] + X[p]*mx[f]
with the per-core row offset folded into C.  Clamping only bites at grid
edges and each case is itself rank-1, so a K=8 bf16 matmul's fp32 PSUM
output IS the clamped result (PE cost scales with N, not K).

Output is stored as bf16 and upcast to f32 on the host: anchor values are
<= 8192 so bf16 rounding is <= 16 absolute (2e-3 of scale, 0.4%
per-element), far inside tolerance, and it halves HBM store traffic
(2.36 MB/core -> ~6.6 us at 358 GB/s).

DMA load balance: an [8, N] SBUF tile at partitions 0-7 maps to only 2 of
the 16 SDMA engines, so a monolithic table load straggles those engines
behind every store (engine-FIFO).  Instead the rhs is split into 4 column
quarters placed at PE quadrant bases 32q (tile_position allows lhsT/rhs
partition bases 0/32/64/96): q0/q1 load on engines {0,2}, q2/q3 on {1,3},
in 3 pieces per quadrant ordered so data always leads PE consumption.
All loads and stores are HWDGE on the SP ring (no gpsimd SWDGE: ~1 us
first-byte and it queues behind framework memsets).

Per chunk the PSUM result is cast-copied (f32 -> bf16) to SBUF alternating
DVE / ACT so the copy never gates the DMA stream; contiguous HWDGE stores
on SP drain 8 groups sized >= 512 B per partition descriptor.
"""

import numpy as np
import ml_dtypes

H = 512
W = 512
N_CORES = 8
ROWS_PER_CORE = H // N_CORES  # 64
P = 128                       # partitions = (y_rel, x_half)
XW = W // 2                   # 256 x-positions per partition
SR = 9                        # scale x ratio combos
FREE = XW * SR * 4            # 9216 values per partition
K = 8                         # bf16 contraction: C1,C2,Ygrid,X + 4 edge fixups
MM_N = 512                    # matmul free-dim (one PSUM bank)

# PE quadrant groups: AP base partitions may only be 0/32/64, so the rhs is
# split in thirds sized to balance the SDMA engine pairs (bases 0 and 32 both
# map to engines {0,2}; base 64 maps to {1,3}): (gbase, dram col off, ncols)
QUADS = ((0, 0, 2304), (2304, 2432, 2304), (4608, 4864, 4608))
TAB_COLS = 9600               # 3 packed quadrants: (128+2304)*2 + (128+4608)
TAB_SB = P + 4608             # SBUF table tile width (widest quadrant)
# chunks in PE order: (quad idx, local col off, size)
CHUNKS = ((0, 0, 256), (0, 256, 1024), (0, 1280, 1024),
          (1, 0, 256), (1, 256, 1024), (1, 1280, 1024),
          (2, 0, 256), (2, 256, 1024), (2, 1280, 1024),
          (2, 2304, 256), (2, 2560, 1024), (2, 3584, 1024))
# load pieces per quadrant in packed cols (incl the 128 lhsT cols); piece
# boundaries align with matmul boundaries
PIECES = (((0, 896), (896, 1920), (1920, 2432)),
          ((0, 896), (896, 1920), (1920, 2432)),
          ((0, 896), (896, 1920), (1920, 2688), (2688, 3712), (3712, 4736)))
# issue order: pair {0,2} streams q0 then q1, pair {1,3} streams q2
LOAD_ORDER = ((0, 0), (2, 0), (0, 1), (2, 1), (0, 2), (2, 2),
              (1, 0), (2, 3), (1, 1), (2, 4), (1, 2))
# stores grouped over chunk ids; every group >= 512 B/partition in bf16
STORE_GROUPS = ((0,), (1,), (2, 3), (4, 5), (6, 7), (8, 9), (10,), (11,))

_cache = {}


def _bf16_split(v, n):
    """Split f64 vector v into n bf16 addends, most-significant first."""
    parts = []
    rem = v.copy()
    for _ in range(n):
        p = rem.astype(ml_dtypes.bfloat16)
        parts.append(p)
        rem = rem - p.astype(np.float64)
    return parts


def _half_sizes():
    """hw, hh as (3,3) f32, matching the reference's jnp ops on this backend."""
    import jax.numpy as jnp

    scales = jnp.asarray((0.5, 1.0, 2.0), dtype=jnp.float32)
    ratios = jnp.asarray((0.5, 1.0, 2.0), dtype=jnp.float32)
    sqrt_r = jnp.sqrt(ratios)
    aw = 16.0 * scales[:, None] * sqrt_r[None, :]
    ah = 16.0 * scales[:, None] / sqrt_r[None, :]
    hw = np.asarray(aw / 2, dtype=np.float32)
    hh = np.asarray(ah / 2, dtype=np.float32)
    return hw, hh


def _tables():
    """Per-core packed bf16 input (QN, K, QPACK): quadrant q holds the lhsT
    columns then rhs columns [QCOLS*q : QCOLS*(q+1)]."""
    hw, hh = _half_sizes()
    off = np.stack([-hw, -hh, hw, hh], axis=-1).reshape(36).astype(np.float64)
    isx = np.tile(np.array([1.0, 0.0, 1.0, 0.0]), SR)  # c parity: x-coords even
    x_rel = np.arange(XW, dtype=np.float64)
    base = 8.0 + 16.0 * x_rel[:, None] * isx[None, :]  # (XW, 36)
    mx = np.broadcast_to(isx, (XW, 36)).reshape(FREE)
    my = 1.0 - mx
    my_b = my.astype(ml_dtypes.bfloat16)
    mx_b = mx.astype(ml_dtypes.bfloat16)

    p = np.arange(P)
    Ygrid = (16.0 * (p // 2)).astype(ml_dtypes.bfloat16)   # exact
    X = (4096.0 * (p % 2)).astype(ml_dtypes.bfloat16)      # exact
    ones = np.ones(P, ml_dtypes.bfloat16)
    Iy0 = (p < 2).astype(ml_dtypes.bfloat16)               # y == 0 rows
    Iyt = (p >= P - 2).astype(ml_dtypes.bfloat16)          # y == 511 rows
    Iev = (1 - p % 2).astype(ml_dtypes.bfloat16)           # x_half == 0
    Iod = (p % 2).astype(ml_dtypes.bfloat16)               # x_half == 1

    f = np.arange(FREE)
    cpos = f % 4
    hh_f = hh.reshape(9)[(f // 4) % 9].astype(np.float64)
    hw_f = hw.reshape(9)[(f // 4) % 9].astype(np.float64)

    lhsT = np.stack([ones, ones, Ygrid, X, Iy0, Iyt, Iev, Iod])  # (K, P)

    packed = np.zeros((N_CORES, K, TAB_COLS), ml_dtypes.bfloat16)
    for c in range(N_CORES):
        # fold the per-core row offset into C's y-columns
        Cc = (base + off[None, :] + 1024.0 * c * (1.0 - isx)[None, :]).reshape(FREE)
        C1, C2 = _bf16_split(Cc, 2)
        Cb = C1.astype(np.float64) + C2.astype(np.float64)
        # edge fixups: exact clamped value minus the unclamped rank-3 sum
        ey0 = np.where((cpos == 1) & (c == 0) & (8 - hh_f < 0), -Cb, 0.0)
        eyt = np.where(
            (cpos == 3) & (c == N_CORES - 1) & (8184 + hh_f > 8192),
            8192.0 - (Cb + 1008.0), 0.0,
        )
        exl = np.where((cpos == 0) & (f < 36) & (8 - hw_f < 0), -Cb, 0.0)
        exr = np.where(
            (cpos == 2) & (f >= FREE - 36) & (8184 + hw_f > 8192),
            8192.0 - (Cb + 4096.0), 0.0,
        )
        rhs = np.stack([
            C1, C2, my_b, mx_b,
            ey0.astype(ml_dtypes.bfloat16), eyt.astype(ml_dtypes.bfloat16),
            exl.astype(ml_dtypes.bfloat16), exr.astype(ml_dtypes.bfloat16),
        ])  # (K, FREE)
        for gbase, qoff, ncols in QUADS:
            packed[c, :, qoff:qoff + P] = lhsT
            packed[c, :, qoff + P:qoff + P + ncols] = rhs[:, gbase:gbase + ncols]
    return packed


def build_nc():
    import concourse.bacc as bacc
    import concourse.mybir as mybir
    import concourse.tile as tile

    nc = bacc.Bacc(None)
    tabs_d = nc.declare_dram_parameter("tabs", [K, TAB_COLS], mybir.dt.bfloat16,
                                       isOutput=False)
    out_d = nc.declare_dram_parameter("out", [P, FREE], mybir.dt.bfloat16,
                                      isOutput=True)

    def chunk_glo(ci):
        q, llo, size = CHUNKS[ci]
        return QUADS[q][0] + llo

    with tile.TileContext(nc) as tc:
        with (
            tc.tile_pool(name="const", bufs=1) as cpool,
            tc.tile_pool(name="osb", bufs=1) as osb,
            tc.tile_pool(name="psb", bufs=3, space="PSUM") as psb,
            tc.tile_pool(name="pss", bufs=2, space="PSUM") as pss,
        ):
            tabs = cpool.tile([P, TAB_SB], mybir.dt.bfloat16)
            for q, pi in LOAD_ORDER:
                a, b = PIECES[q][pi]
                qoff = QUADS[q][1]
                nc.sync.dma_start(tabs[32 * q:32 * q + K, a:b],
                                  tabs_d[:, qoff + a:qoff + b])
            obuf = osb.tile([P, FREE], mybir.dt.bfloat16)
            for ci, (q, llo, size) in enumerate(CHUNKS):
                glo = QUADS[q][0] + llo
                pool = psb if size > 512 else pss
                acc = pool.tile([P, size], mybir.dt.float32,
                                tag="accb" if size > 512 else "accs")
                lhsT = tabs[32 * q:32 * q + K, :P]
                for m0 in range(0, size, MM_N):
                    n = min(MM_N, size - m0)
                    rhs = tabs[32 * q:32 * q + K,
                               P + llo + m0: P + llo + m0 + n]
                    nc.tensor.matmul(acc[:, m0:m0 + n], lhsT, rhs)
                o = obuf[:, glo:glo + size]
                if ci % 2 == 0:
                    nc.vector.tensor_copy(o, acc[:])
                else:
                    nc.scalar.copy(o, acc[:])
                # emit the store whose group this chunk completes
                for grp in STORE_GROUPS:
                    if grp[-1] == ci:
                        slo = chunk_glo(grp[0])
                        nc.sync.dma_start(out_d[:, slo:glo + size],
                                          obuf[:, slo:glo + size])
    nc.compile()
    return nc


def kernel(feature_map: np.ndarray) -> np.ndarray:
    from concourse.bass_utils import run_bass_kernel_spmd

    if "tables" not in _cache:
        _cache["tables"] = _tables()
    packed = _cache["tables"]
    if "nc" not in _cache:
        _cache["nc"] = build_nc()
    nc = _cache["nc"]

    in_maps = [{"tabs": packed[c]} for c in range(N_CORES)]
    res = run_bass_kernel_spmd(nc, in_maps, core_ids=list(range(N_CORES)))
    return np.concatenate(
        [np.asarray(res.results[c]["out"], dtype=np.float32).reshape(-1, 4)
         for c in range(N_CORES)],
        axis=0,
    )


# revision 10
# speedup vs baseline: 1.0592x; 1.0592x over previous
"""AnchorGenerator on 8 TRN2 NeuronCores.

The reference output depends only on H=W=512 (feature_map values are unused):
for each (y, x, s, r) the anchor row is
    [max(16x+8-hw, 0), max(16y+8-hh, 0), min(16x+8+hw, 8192), min(16y+8+hh, 8192)]
with hw/hh the 3x3 half-width/height tables.

Sharding: 64 grid rows per core. Per core the flat (294912, 4) output slab
is exactly a [128, 9216] SBUF tile in partition-major order, with
partition p = (y_rel, x_half) and free index f = x_rel*36 + (s*3+r)*4 + c.

The unclamped value decomposes as a low-rank product
    v[p, f] = C[f] + Ygrid[p]*my[f] + X[p]*mx[f]
with the per-core row offset folded into C.  Clamping only bites at grid
edges and each case is itself rank-1, so a K=8 bf16 matmul's fp32 PSUM
output IS the clamped result (PE cost scales with N, not K).

Output is stored as bf16 and upcast to f32 on the host: anchor values are
<= 8192 so bf16 rounding is <= 16 absolute (2e-3 of scale, 0.4%
per-element), far inside tolerance, and it halves HBM store traffic
(2.36 MB/core -> ~6.6 us at 358 GB/s).

DMA load balance: an [8, N] SBUF tile at partitions 0-7 maps to only 2 of
the 16 SDMA engines, so a monolithic table load straggles those engines
behind every store (engine-FIFO).  Instead the rhs is split across the 3
legal PE partition bases (0/32/64; AP base_partition rejects 96), which
spreads the load over 8 engines, with the base-64 quadrant double-width to
balance bytes.  Loads ride the gpsimd SWDGE ring (free after the framework
memsets) because each dma_start costs ~600 ns of sequencer issue time and
the SP sequencer must be free to issue stores the moment copies land.

Per chunk the PSUM result is cast-copied (f32 -> bf16) to SBUF alternating
DVE / ACT so the copy never gates the DMA stream; contiguous HWDGE stores
on SP drain 8 groups sized >= 512 B per partition descriptor.
"""

import numpy as np
import ml_dtypes

H = 512
W = 512
N_CORES = 8
ROWS_PER_CORE = H // N_CORES  # 64
P = 128                       # partitions = (y_rel, x_half)
XW = W // 2                   # 256 x-positions per partition
SR = 9                        # scale x ratio combos
FREE = XW * SR * 4            # 9216 values per partition
K = 8                         # bf16 contraction: C1,C2,Ygrid,X + 4 edge fixups
MM_N = 512                    # matmul free-dim (one PSUM bank)

# PE quadrant groups: AP base partitions may only be 0/32/64, so the rhs is
# split in thirds sized to balance the SDMA engine pairs (bases 0 and 32 both
# map to engines {0,2}; base 64 maps to {1,3}): (gbase, dram col off, ncols)
QUADS = ((0, 0, 2304), (2304, 2432, 2304), (4608, 4864, 4608))
TAB_COLS = 9600               # 3 packed quadrants: (128+2304)*2 + (128+4608)
TAB_SB = P + 4608             # SBUF table tile width (widest quadrant)
# chunks in PE order: (quad idx, local col off, size)
CHUNKS = ((0, 0, 256), (0, 256, 1024), (0, 1280, 1024),
          (1, 0, 256), (1, 256, 1024), (1, 1280, 1024),
          (2, 0, 256), (2, 256, 1024), (2, 1280, 1024),
          (2, 2304, 256), (2, 2560, 1024), (2, 3584, 1024))
# load pieces (quad, packed col lo, hi) in PE-consumption order; every
# dma_start costs ~600 ns of sequencer issue time, so keep the count small --
# just a split first piece so PE starts early.  Boundaries align with matmul
# boundaries (local 768 / 2560 + the 128 lhsT cols).
LOADS = ((0, 0, 896), (0, 896, 2432), (1, 0, 2432),
         (2, 0, 2688), (2, 2688, 4736))
# stores grouped over chunk ids; every group >= 512 B/partition in bf16
STORE_GROUPS = ((0,), (1,), (2, 3), (4, 5), (6, 7), (8, 9), (10,), (11,))

_cache = {}


def _bf16_split(v, n):
    """Split f64 vector v into n bf16 addends, most-significant first."""
    parts = []
    rem = v.copy()
    for _ in range(n):
        p = rem.astype(ml_dtypes.bfloat16)
        parts.append(p)
        rem = rem - p.astype(np.float64)
    return parts


def _half_sizes():
    """hw, hh as (3,3) f32, matching the reference's jnp ops on this backend."""
    import jax.numpy as jnp

    scales = jnp.asarray((0.5, 1.0, 2.0), dtype=jnp.float32)
    ratios = jnp.asarray((0.5, 1.0, 2.0), dtype=jnp.float32)
    sqrt_r = jnp.sqrt(ratios)
    aw = 16.0 * scales[:, None] * sqrt_r[None, :]
    ah = 16.0 * scales[:, None] / sqrt_r[None, :]
    hw = np.asarray(aw / 2, dtype=np.float32)
    hh = np.asarray(ah / 2, dtype=np.float32)
    return hw, hh


def _tables():
    """Per-core packed bf16 input (QN, K, QPACK): quadrant q holds the lhsT
    columns then rhs columns [QCOLS*q : QCOLS*(q+1)]."""
    hw, hh = _half_sizes()
    off = np.stack([-hw, -hh, hw, hh], axis=-1).reshape(36).astype(np.float64)
    isx = np.tile(np.array([1.0, 0.0, 1.0, 0.0]), SR)  # c parity: x-coords even
    x_rel = np.arange(XW, dtype=np.float64)
    base = 8.0 + 16.0 * x_rel[:, None] * isx[None, :]  # (XW, 36)
    mx = np.broadcast_to(isx, (XW, 36)).reshape(FREE)
    my = 1.0 - mx
    my_b = my.astype(ml_dtypes.bfloat16)
    mx_b = mx.astype(ml_dtypes.bfloat16)

    p = np.arange(P)
    Ygrid = (16.0 * (p // 2)).astype(ml_dtypes.bfloat16)   # exact
    X = (4096.0 * (p % 2)).astype(ml_dtypes.bfloat16)      # exact
    ones = np.ones(P, ml_dtypes.bfloat16)
    Iy0 = (p < 2).astype(ml_dtypes.bfloat16)               # y == 0 rows
    Iyt = (p >= P - 2).astype(ml_dtypes.bfloat16)          # y == 511 rows
    Iev = (1 - p % 2).astype(ml_dtypes.bfloat16)           # x_half == 0
    Iod = (p % 2).astype(ml_dtypes.bfloat16)               # x_half == 1

    f = np.arange(FREE)
    cpos = f % 4
    hh_f = hh.reshape(9)[(f // 4) % 9].astype(np.float64)
    hw_f = hw.reshape(9)[(f // 4) % 9].astype(np.float64)

    lhsT = np.stack([ones, ones, Ygrid, X, Iy0, Iyt, Iev, Iod])  # (K, P)

    packed = np.zeros((N_CORES, K, TAB_COLS), ml_dtypes.bfloat16)
    for c in range(N_CORES):
        # fold the per-core row offset into C's y-columns
        Cc = (base + off[None, :] + 1024.0 * c * (1.0 - isx)[None, :]).reshape(FREE)
        C1, C2 = _bf16_split(Cc, 2)
        Cb = C1.astype(np.float64) + C2.astype(np.float64)
        # edge fixups: exact clamped value minus the unclamped rank-3 sum
        ey0 = np.where((cpos == 1) & (c == 0) & (8 - hh_f < 0), -Cb, 0.0)
        eyt = np.where(
            (cpos == 3) & (c == N_CORES - 1) & (8184 + hh_f > 8192),
            8192.0 - (Cb + 1008.0), 0.0,
        )
        exl = np.where((cpos == 0) & (f < 36) & (8 - hw_f < 0), -Cb, 0.0)
        exr = np.where(
            (cpos == 2) & (f >= FREE - 36) & (8184 + hw_f > 8192),
            8192.0 - (Cb + 4096.0), 0.0,
        )
        rhs = np.stack([
            C1, C2, my_b, mx_b,
            ey0.astype(ml_dtypes.bfloat16), eyt.astype(ml_dtypes.bfloat16),
            exl.astype(ml_dtypes.bfloat16), exr.astype(ml_dtypes.bfloat16),
        ])  # (K, FREE)
        for gbase, qoff, ncols in QUADS:
            packed[c, :, qoff:qoff + P] = lhsT
            packed[c, :, qoff + P:qoff + P + ncols] = rhs[:, gbase:gbase + ncols]
    return packed


def build_nc():
    import concourse.bacc as bacc
    import concourse.mybir as mybir
    import concourse.tile as tile

    nc = bacc.Bacc(None)
    tabs_d = nc.declare_dram_parameter("tabs", [K, TAB_COLS], mybir.dt.bfloat16,
                                       isOutput=False)
    out_d = nc.declare_dram_parameter("out", [P, FREE], mybir.dt.bfloat16,
                                      isOutput=True)

    def chunk_glo(ci):
        q, llo, size = CHUNKS[ci]
        return QUADS[q][0] + llo

    with tile.TileContext(nc) as tc:
        with (
            tc.tile_pool(name="const", bufs=1) as cpool,
            tc.tile_pool(name="osb", bufs=1) as osb,
            tc.tile_pool(name="psb", bufs=3, space="PSUM") as psb,
            tc.tile_pool(name="pss", bufs=2, space="PSUM") as pss,
        ):
            tabs = cpool.tile([P, TAB_SB], mybir.dt.bfloat16)
            # loads ride the free gpsimd SWDGE ring so the SP sequencer is
            # dedicated to store issue
            for q, a, b in LOADS:
                qoff = QUADS[q][1]
                nc.gpsimd.dma_start(tabs[32 * q:32 * q + K, a:b],
                                    tabs_d[:, qoff + a:qoff + b])
            obuf = osb.tile([P, FREE], mybir.dt.bfloat16)
            for ci, (q, llo, size) in enumerate(CHUNKS):
                glo = QUADS[q][0] + llo
                pool = psb if size > 512 else pss
                acc = pool.tile([P, size], mybir.dt.float32,
                                tag="accb" if size > 512 else "accs")
                lhsT = tabs[32 * q:32 * q + K, :P]
                for m0 in range(0, size, MM_N):
                    n = min(MM_N, size - m0)
                    rhs = tabs[32 * q:32 * q + K,
                               P + llo + m0: P + llo + m0 + n]
                    nc.tensor.matmul(acc[:, m0:m0 + n], lhsT, rhs)
                o = obuf[:, glo:glo + size]
                if ci % 2 == 0:
                    nc.vector.tensor_copy(o, acc[:])
                else:
                    nc.scalar.copy(o, acc[:])
                # emit the store whose group this chunk completes
                for grp in STORE_GROUPS:
                    if grp[-1] == ci:
                        slo = chunk_glo(grp[0])
                        nc.sync.dma_start(out_d[:, slo:glo + size],
                                          obuf[:, slo:glo + size])
    nc.compile()
    return nc


def kernel(feature_map: np.ndarray) -> np.ndarray:
    from concourse.bass_utils import run_bass_kernel_spmd

    if "tables" not in _cache:
        _cache["tables"] = _tables()
    packed = _cache["tables"]
    if "nc" not in _cache:
        _cache["nc"] = build_nc()
    nc = _cache["nc"]

    in_maps = [{"tabs": packed[c]} for c in range(N_CORES)]
    res = run_bass_kernel_spmd(nc, in_maps, core_ids=list(range(N_CORES)))
    return np.concatenate(
        [np.asarray(res.results[c]["out"], dtype=np.float32).reshape(-1, 4)
         for c in range(N_CORES)],
        axis=0,
    )


# revision 12
# speedup vs baseline: 1.0680x; 1.0083x over previous
"""AnchorGenerator on 8 TRN2 NeuronCores.

The reference output depends only on H=W=512 (feature_map values are unused):
for each (y, x, s, r) the anchor row is
    [max(16x+8-hw, 0), max(16y+8-hh, 0), min(16x+8+hw, 8192), min(16y+8+hh, 8192)]
with hw/hh the 3x3 half-width/height tables.

Sharding: 64 grid rows per core. Per core the flat (294912, 4) output slab
is exactly a [128, 9216] SBUF tile in partition-major order, with
partition p = (y_rel, x_half) and free index f = x_rel*36 + (s*3+r)*4 + c.

The unclamped value decomposes as a low-rank product
    v[p, f] = C[f] + Ygrid[p]*my[f] + X[p]*mx[f]
with the per-core row offset folded into C.  Clamping only bites at grid
edges and each case is itself rank-1, so a K=8 bf16 matmul's fp32 PSUM
output IS the clamped result (PE cost scales with N, not K).

Output is stored as bf16 and upcast to f32 on the host: anchor values are
<= 8192 so bf16 rounding is <= 16 absolute (2e-3 of scale, 0.4%
per-element), far inside tolerance, and it halves HBM store traffic
(2.36 MB/core -> ~6.6 us at 358 GB/s).

DMA load balance: an [8, N] SBUF tile at partitions 0-7 maps to only 2 of
the 16 SDMA engines, so a monolithic table load straggles those engines
behind every store (engine-FIFO).  Instead the rhs is split across the 3
legal PE partition bases (0/32/64; AP base_partition rejects 96), which
spreads the load over 8 engines, with the base-64 quadrant double-width to
balance bytes.  Loads ride the gpsimd SWDGE ring (free after the framework
memsets) because each dma_start costs ~600 ns of sequencer issue time and
the SP sequencer must be free to issue stores the moment copies land.

Per chunk the PSUM result is cast-copied (f32 -> bf16) to SBUF alternating
DVE / ACT so the copy never gates the DMA stream; contiguous HWDGE stores
on SP drain 8 groups sized >= 512 B per partition descriptor.
"""

import numpy as np
import ml_dtypes

H = 512
W = 512
N_CORES = 8
ROWS_PER_CORE = H // N_CORES  # 64
P = 128                       # partitions = (y_rel, x_half)
XW = W // 2                   # 256 x-positions per partition
SR = 9                        # scale x ratio combos
FREE = XW * SR * 4            # 9216 values per partition
K = 8                         # bf16 contraction: C1,C2,Ygrid,X + 4 edge fixups
MM_N = 512                    # matmul free-dim (one PSUM bank)

# PE quadrant groups: AP base partitions may only be 0/32/64, so the rhs is
# split in thirds sized to balance the SDMA engine pairs (bases 0 and 32 both
# map to engines {0,2}; base 64 maps to {1,3}): (gbase, dram col off, ncols)
QUADS = ((0, 0, 2304), (2304, 2432, 2304), (4608, 4864, 4608))
TAB_COLS = 9600               # 3 packed quadrants: (128+2304)*2 + (128+4608)
TAB_SB = P + 4608             # SBUF table tile width (widest quadrant)
# chunks in PE order: (quad idx, local col off, size); the tail is split
# into two 512s so the final two copies run on both engines in parallel
CHUNKS = ((0, 0, 256), (0, 256, 1024), (0, 1280, 1024),
          (1, 0, 256), (1, 256, 1024), (1, 1280, 1024),
          (2, 0, 256), (2, 256, 1024), (2, 1280, 1024),
          (2, 2304, 256), (2, 2560, 1024), (2, 3584, 512), (2, 4096, 512))
# quadrant 0 (incl its lhsT) loads via HWDGE on SP in the MAIN block, before
# tile entry, gated by a PE wait_ge: PE starts ~2.4 us in with 1.9 us of q0
# work queued.  The rest loads via in-tile SWDGE on the free gpsimd ring
# (quad, packed col lo, hi); boundaries align with matmul boundaries.
REST_LOADS = ((1, 0, 2432), (2, 0, 2688), (2, 2688, 4736))
# stores grouped over chunk ids; every group >= 512 B/partition in bf16
STORE_GROUPS = ((0,), (1,), (2, 3), (4, 5), (6, 7), (8, 9), (10, 11), (12,))

_cache = {}


def _bf16_split(v, n):
    """Split f64 vector v into n bf16 addends, most-significant first."""
    parts = []
    rem = v.copy()
    for _ in range(n):
        p = rem.astype(ml_dtypes.bfloat16)
        parts.append(p)
        rem = rem - p.astype(np.float64)
    return parts


def _half_sizes():
    """hw, hh as (3,3) f32, matching the reference's jnp ops on this backend."""
    import jax.numpy as jnp

    scales = jnp.asarray((0.5, 1.0, 2.0), dtype=jnp.float32)
    ratios = jnp.asarray((0.5, 1.0, 2.0), dtype=jnp.float32)
    sqrt_r = jnp.sqrt(ratios)
    aw = 16.0 * scales[:, None] * sqrt_r[None, :]
    ah = 16.0 * scales[:, None] / sqrt_r[None, :]
    hw = np.asarray(aw / 2, dtype=np.float32)
    hh = np.asarray(ah / 2, dtype=np.float32)
    return hw, hh


def _tables():
    """Per-core packed bf16 input (QN, K, QPACK): quadrant q holds the lhsT
    columns then rhs columns [QCOLS*q : QCOLS*(q+1)]."""
    hw, hh = _half_sizes()
    off = np.stack([-hw, -hh, hw, hh], axis=-1).reshape(36).astype(np.float64)
    isx = np.tile(np.array([1.0, 0.0, 1.0, 0.0]), SR)  # c parity: x-coords even
    x_rel = np.arange(XW, dtype=np.float64)
    base = 8.0 + 16.0 * x_rel[:, None] * isx[None, :]  # (XW, 36)
    mx = np.broadcast_to(isx, (XW, 36)).reshape(FREE)
    my = 1.0 - mx
    my_b = my.astype(ml_dtypes.bfloat16)
    mx_b = mx.astype(ml_dtypes.bfloat16)

    p = np.arange(P)
    Ygrid = (16.0 * (p // 2)).astype(ml_dtypes.bfloat16)   # exact
    X = (4096.0 * (p % 2)).astype(ml_dtypes.bfloat16)      # exact
    ones = np.ones(P, ml_dtypes.bfloat16)
    Iy0 = (p < 2).astype(ml_dtypes.bfloat16)               # y == 0 rows
    Iyt = (p >= P - 2).astype(ml_dtypes.bfloat16)          # y == 511 rows
    Iev = (1 - p % 2).astype(ml_dtypes.bfloat16)           # x_half == 0
    Iod = (p % 2).astype(ml_dtypes.bfloat16)               # x_half == 1

    f = np.arange(FREE)
    cpos = f % 4
    hh_f = hh.reshape(9)[(f // 4) % 9].astype(np.float64)
    hw_f = hw.reshape(9)[(f // 4) % 9].astype(np.float64)

    lhsT = np.stack([ones, ones, Ygrid, X, Iy0, Iyt, Iev, Iod])  # (K, P)

    packed = np.zeros((N_CORES, K, TAB_COLS), ml_dtypes.bfloat16)
    for c in range(N_CORES):
        # fold the per-core row offset into C's y-columns
        Cc = (base + off[None, :] + 1024.0 * c * (1.0 - isx)[None, :]).reshape(FREE)
        C1, C2 = _bf16_split(Cc, 2)
        Cb = C1.astype(np.float64) + C2.astype(np.float64)
        # edge fixups: exact clamped value minus the unclamped rank-3 sum
        ey0 = np.where((cpos == 1) & (c == 0) & (8 - hh_f < 0), -Cb, 0.0)
        eyt = np.where(
            (cpos == 3) & (c == N_CORES - 1) & (8184 + hh_f > 8192),
            8192.0 - (Cb + 1008.0), 0.0,
        )
        exl = np.where((cpos == 0) & (f < 36) & (8 - hw_f < 0), -Cb, 0.0)
        exr = np.where(
            (cpos == 2) & (f >= FREE - 36) & (8184 + hw_f > 8192),
            8192.0 - (Cb + 4096.0), 0.0,
        )
        rhs = np.stack([
            C1, C2, my_b, mx_b,
            ey0.astype(ml_dtypes.bfloat16), eyt.astype(ml_dtypes.bfloat16),
            exl.astype(ml_dtypes.bfloat16), exr.astype(ml_dtypes.bfloat16),
        ])  # (K, FREE)
        for gbase, qoff, ncols in QUADS:
            packed[c, :, qoff:qoff + P] = lhsT
            packed[c, :, qoff + P:qoff + P + ncols] = rhs[:, gbase:gbase + ncols]
    return packed


def build_nc():
    import contextlib

    import concourse.bacc as bacc
    import concourse.mybir as mybir
    import concourse.tile as tile

    nc = bacc.Bacc(None)
    tabs_d = nc.declare_dram_parameter("tabs", [K, TAB_COLS], mybir.dt.bfloat16,
                                       isOutput=False)
    out_d = nc.declare_dram_parameter("out", [P, FREE], mybir.dt.bfloat16,
                                      isOutput=True)

    def chunk_glo(ci):
        q, llo, size = CHUNKS[ci]
        return QUADS[q][0] + llo

    # q0 (lhsT + rhs cols) loads in MAIN via HWDGE on the idle SP sequencer;
    # its transfer overlaps the framework preamble and tile-entry barrier,
    # and a PE wait_ge (outside the scheduled tile block) holds the matmuls.
    es = contextlib.ExitStack()
    headbuf = es.enter_context(
        nc.sbuf_tensor("tabs_q0", [K, 2432], mybir.dt.bfloat16)
    )
    _cache.setdefault("es", []).append(es)  # keep the allocation alive
    s_head = nc.alloc_semaphore("tab_head")
    nc.sync.dma_start(headbuf[:, :], tabs_d[:, :2432]).then_inc(s_head, 16)
    nc.tensor.wait_ge(s_head, 16)

    with tile.TileContext(nc) as tc:
        with (
            tc.tile_pool(name="const", bufs=1) as cpool,
            tc.tile_pool(name="osb", bufs=1) as osb,
            tc.tile_pool(name="psb", bufs=3, space="PSUM") as psb,
            tc.tile_pool(name="pss", bufs=2, space="PSUM") as pss,
        ):
            rest = cpool.tile([P, TAB_SB], mybir.dt.bfloat16)
            for q, a, b in REST_LOADS:
                qoff = QUADS[q][1]
                nc.gpsimd.dma_start(rest[32 * q:32 * q + K, a:b],
                                    tabs_d[:, qoff + a:qoff + b])
            obuf = osb.tile([P, FREE], mybir.dt.bfloat16)
            for ci, (q, llo, size) in enumerate(CHUNKS):
                glo = QUADS[q][0] + llo
                pool = psb if size > 512 else pss
                acc = pool.tile([P, size], mybir.dt.float32,
                                tag="accb" if size > 512 else "accs")
                tab = headbuf if q == 0 else rest
                lhsT = tab[32 * q:32 * q + K, :P]
                for m0 in range(0, size, MM_N):
                    n = min(MM_N, size - m0)
                    rhs = tab[32 * q:32 * q + K,
                              P + llo + m0: P + llo + m0 + n]
                    nc.tensor.matmul(acc[:, m0:m0 + n], lhsT, rhs)
                o = obuf[:, glo:glo + size]
                if ci % 2 == 0:
                    nc.scalar.copy(o, acc[:])
                else:
                    nc.vector.tensor_copy(o, acc[:])
                # emit the store whose group this chunk completes
                for grp in STORE_GROUPS:
                    if grp[-1] == ci:
                        slo = chunk_glo(grp[0])
                        nc.sync.dma_start(out_d[:, slo:glo + size],
                                          obuf[:, slo:glo + size])
    nc.compile()
    return nc


def kernel(feature_map: np.ndarray) -> np.ndarray:
    from concourse.bass_utils import run_bass_kernel_spmd

    if "tables" not in _cache:
        _cache["tables"] = _tables()
    packed = _cache["tables"]
    if "nc" not in _cache:
        _cache["nc"] = build_nc()
    nc = _cache["nc"]

    in_maps = [{"tabs": packed[c]} for c in range(N_CORES)]
    res = run_bass_kernel_spmd(nc, in_maps, core_ids=list(range(N_CORES)))
    return np.concatenate(
        [np.asarray(res.results[c]["out"], dtype=np.float32).reshape(-1, 4)
         for c in range(N_CORES)],
        axis=0,
    )
